# revision 16
# baseline (speedup 1.0000x reference)
"""Trainium2 Bass kernel for nn_CLIPCrossProductClassifier.

Math:  y[b,h] = sum_{i,j} img_n[b,i] * txt_n[b,j] * W1r[i,j,h]
       logits = relu(y + b1) @ W2 + b2
with img_n/txt_n the L2-normalized embeddings and W1r = W1.reshape(D,D,H).

Sharding: contraction-parallel over i (rows of the bilinear form). Each of
the 8 cores owns 64 values of i (a [64*D, H] row-slice of W1, 32 MB in f16)
and computes a partial y_c[b,h] = sum_{i in I_c, j} ... . The partials are
summed on the host (8 x 1 MB), followed by the tiny bias/ReLU/[512x1]
projection (0.5 MFLOP of the 137 GFLOP total).

Per-core device schedule (b on PSUM partitions so the img scale is a
per-partition scalar):
  for i in 64:                       # i local to the core
    for b_blk in 4:                  # batch in blocks of 128
      psum[b,h] = sum_{j_chunk in 4} txtT[j_chunk, b_blk].T @ W1[i, j_chunk, :, :]
      acc[b_blk] = psum * img[b_blk, i] + acc[b_blk]   # one fused DVE op
W1 streams through as the moving operand in f16 (1 cycle/row at N=512 =
full PE rate, half the HBM bytes of f32); txtT is the stationary operand
(f16 -> fast weight load). W1 is pre-scaled by 512 on the host (sigma ~1,
no f16 subnormals); the 1/512 is folded into the per-partition img scale.

DMA layout: every SBUF row is >= 2 KB so the DMA moves full-size packets
(the f32 baseline was packet-rate-bound at ~1 KB/packet):
 - W1 per i as one [128, 2048] f16 tile (row r = W1[i, {r,128+r,256+r,384+r}, :])
 - txt as one [128, 2048] f16 tile (row r = txtT[{r,128+r,256+r,384+r}, :])
 - img as one [128, 256] f32 tile (row r = img[{r,128+r,256+r,384+r}, :]/512)

Epilogue is a single fused InstTensorScalarPtr per (i, b_blk) on the Vector
engine: acc = (psum * img_scalar) + acc, reading PSUM directly.

Startup: dummy matmuls on a zeroed tile keep the PE HAM un-throttled
(2.4 GHz) while the first input DMAs land; txt is first in the DMA queues.
"""

import numpy as np

import concourse.bass as bass
import concourse.tile as tile
from concourse import bacc, mybir
from concourse.bass_utils import run_bass_kernel_spmd

B, D, H = 512, 512, 512
N_CORES = 8
I_PER_CORE = D // N_CORES          # 64
N_BBLK = B // 128                  # 4
N_JCHUNK = D // 128                # 4
EPS = 1e-12
W1_SCALE = 512.0                   # power of two: exact in fp, folded into img

F32 = mybir.dt.float32
F16 = mybir.dt.float16

N_WARM = 12                        # dummy matmuls to keep HAM warm at start

_CACHE = {}


def _l2norm(x: np.ndarray) -> np.ndarray:
    n = np.sqrt(np.sum(x * x, axis=1, keepdims=True, dtype=np.float32))
    return (x / np.maximum(n, np.float32(EPS))).astype(np.float32)


def build_nc():
    """Build the per-core Bass program (SPMD: same program, per-core data)."""
    nc = bacc.Bacc(
        "TRN2",
        target_bir_lowering=False,
        debug=False,
        num_devices=N_CORES,
    )

    txt_p = nc.dram_tensor("txt_p", [128, N_JCHUNK * B], F16, kind="ExternalInput").ap()
    img_p = nc.dram_tensor(
        "img_p", [128, N_BBLK * I_PER_CORE], F32, kind="ExternalInput"
    ).ap()
    w1_s = nc.dram_tensor(
        "w1_s", [I_PER_CORE, 128, N_JCHUNK * H], F16, kind="ExternalInput"
    ).ap()
    yp = nc.dram_tensor("yp", [B, H], F16, kind="ExternalOutput").ap()

    with tile.TileContext(nc) as tc:
        with (
            tc.tile_pool(name="warm", bufs=1) as warmp,
            tc.tile_pool(name="warmps", bufs=2, space=bass.MemorySpace.PSUM) as warmpsp,
            tc.tile_pool(name="const", bufs=1) as constp,
            tc.tile_pool(name="w1", bufs=8) as w1p,
            tc.tile_pool(name="accs", bufs=1) as accp,
            tc.tile_pool(name="ps", bufs=6, space=bass.MemorySpace.PSUM) as psump,
        ):
            # txt chunk c0 first in the DMA queues, then W1[i=0] quarter by
            # quarter interleaved with the rest of txt: the first matmul only
            # needs txt c0 + W1 q0, and the i=0 c-outer loop below consumes
            # quarters in arrival order.
            txt_sb = constp.tile([128, N_JCHUNK * B], F16, tag="txt", name="txt_sb")
            w1t0 = w1p.tile([128, N_JCHUNK * H], F16, tag="w1", name="w1p0")
            nc.sync.dma_start(txt_sb[:, :B], txt_p[:, :B])
            nc.sync.dma_start(w1t0[:, :H], w1_s[0, :, :H])
            nc.sync.dma_start(txt_sb[:, B:], txt_p[:, B:])
            for q in range(1, N_JCHUNK):
                nc.sync.dma_start(
                    w1t0[:, q * H : (q + 1) * H], w1_s[0, :, q * H : (q + 1) * H]
                )
            # img right after the first slab: the i=0 epilogue needs it to
            # release the i=0 PSUM banks (it is tiny).
            img_sb = constp.tile(
                [128, N_BBLK * I_PER_CORE], F32, tag="img", name="img_sb"
            )
            nc.sync.dma_start(img_sb[:], img_p[:, :])

            # Prefetch the next two W1 slabs before the main loop: i=0's
            # matmuls finish in ~3.5 us and the i=1 slab must already be
            # resident or the PE stalls (and the HAM re-throttles).
            w1_pre = {}
            for i in (1, 2):
                t = w1p.tile([128, N_JCHUNK * H], F16, tag="w1", name=f"w1pre{i}")
                nc.sync.dma_start(t[:], w1_s[i])
                w1_pre[i] = t

            acc = [
                accp.tile([128, H], F32, tag=f"acc{bb}", name=f"acc{bb}")
                for bb in range(N_BBLK)
            ]
            # f16 staging for the final partial: halves the output DMA and
            # keeps the f32 accumulator out of the tail critical path.
            yp_sb = [
                accp.tile([128, H], F16, tag=f"yp{bb}", name=f"yp{bb}")
                for bb in range(N_BBLK)
            ]

            # Warm-up: dummy matmuls on a zeroed tile keep the PE busy (and
            # the HAM un-throttled) while the real input DMAs land. They sit
            # ahead of the real matmuls on the tensor queue and have no data
            # dependencies beyond the one memset.
            wz = warmp.tile([128, 384], F16, tag="wz", name="warm_zero")
            nc.vector.memset(wz[:], 0.0)
            for k in range(N_WARM):
                wps = warmpsp.tile([128, 256], F32, tag="wps")
                nc.tensor.matmul(
                    wps[:], wz[:, :128], wz[:, 128:384], start=True, stop=True
                )

            # i = 0: c-outer so each W1 quarter feeds 4 matmuls while the next
            # quarter's DMA lands (no startup stall on the full slab).
            ps0 = [
                psump.tile([128, H], F32, tag="ps", name=f"ps0_{bb}")
                for bb in range(N_BBLK)
            ]
            for c in range(N_JCHUNK):
                for bb in range(N_BBLK):
                    nc.tensor.matmul(
                        ps0[bb][:],
                        txt_sb[:, c * B + bb * 128 : c * B + (bb + 1) * 128],
                        w1t0[:, c * H : (c + 1) * H],
                        start=(c == 0),
                        stop=(c == N_JCHUNK - 1),
                    )
            for bb in range(N_BBLK):
                sc = img_sb[:, bb * I_PER_CORE : bb * I_PER_CORE + 1]
                # acc = psum * img  (fused, Vector engine)
                nc.vector.tensor_scalar_mul(acc[bb][:], ps0[bb][:], sc)

            for i in range(1, I_PER_CORE):
                if i in w1_pre:
                    w1t = w1_pre[i]
                else:
                    w1t = w1p.tile([128, N_JCHUNK * H], F16, tag="w1", name="w1")
                    nc.sync.dma_start(w1t[:], w1_s[i])
                for bb in range(N_BBLK):
                    ps = psump.tile([128, H], F32, tag="ps")
                    for c in range(N_JCHUNK):
                        nc.tensor.matmul(
                            ps[:],
                            txt_sb[:, c * B + bb * 128 : c * B + (bb + 1) * 128],
                            w1t[:, c * H : (c + 1) * H],
                            start=(c == 0),
                            stop=(c == N_JCHUNK - 1),
                        )
                    sc = img_sb[:, bb * I_PER_CORE + i : bb * I_PER_CORE + i + 1]
                    if i == I_PER_CORE - 1:
                        # Final partial goes straight to the f16 staging tile.
                        nc.vector.scalar_tensor_tensor(
                            yp_sb[bb][:], ps[:], sc, acc[bb][:],
                            mybir.AluOpType.mult, mybir.AluOpType.add,
                        )
                        # Issue the output DMA from the (idle) scalar engine
                        # so the 4 issues don't serialize on the sync queue.
                        nc.scalar.dma_start(
                            yp[bb * 128 : (bb + 1) * 128, :], yp_sb[bb][:]
                        )
                    else:
                        # acc = (psum * img) + acc  (one fused DVE op)
                        nc.vector.scalar_tensor_tensor(
                            acc[bb][:], ps[:], sc, acc[bb][:],
                            mybir.AluOpType.mult, mybir.AluOpType.add,
                        )

    nc.compile()
    return nc


def make_in_maps(image_embeds, text_embeds, W1):
    imgn = _l2norm(np.asarray(image_embeds, np.float32)) * np.float32(1.0 / W1_SCALE)
    txtn = _l2norm(np.asarray(text_embeds, np.float32))
    # txt packed: row r = [txtT[r], txtT[128+r], txtT[256+r], txtT[384+r]]
    txt_t = np.ascontiguousarray(txtn.T).astype(np.float16)          # [D, B]
    txt_p = np.ascontiguousarray(
        txt_t.reshape(N_JCHUNK, 128, B).transpose(1, 0, 2).reshape(128, N_JCHUNK * B)
    )
    W1r = (np.asarray(W1, np.float32).reshape(D, D, H) * np.float32(W1_SCALE)).astype(
        np.float16
    )
    in_maps = []
    for c in range(N_CORES):
        # img packed per core: row r = [img[r, Ic], img[128+r, Ic], ...]
        ic = np.ascontiguousarray(imgn[:, c * I_PER_CORE : (c + 1) * I_PER_CORE])
        img_pk = np.ascontiguousarray(
            ic.reshape(N_BBLK, 128, I_PER_CORE)
            .transpose(1, 0, 2)
            .reshape(128, N_BBLK * I_PER_CORE)
        )
        # W1 per i: row r = [W1[i, r, :], W1[i, 128+r, :], W1[i, 256+r, :], W1[i, 384+r, :]]
        w1c = W1r[c * I_PER_CORE : (c + 1) * I_PER_CORE]             # [64, D, H]
        w1pk = np.ascontiguousarray(
            w1c.reshape(I_PER_CORE, N_JCHUNK, 128, H)
            .transpose(0, 2, 1, 3)
            .reshape(I_PER_CORE, 128, N_JCHUNK * H)
        )
        in_maps.append({"txt_p": txt_p, "img_p": img_pk, "w1_s": w1pk})
    return in_maps


def run_device(in_maps, trace=False, **kw):
    if "nc" not in _CACHE:
        _CACHE["nc"] = build_nc()
    return run_bass_kernel_spmd(
        _CACHE["nc"], in_maps, list(range(N_CORES)), trace=trace, **kw
    )


def finish_host(results, b1, W2, b2):
    Y = np.zeros((B, H), np.float32)
    for c in range(N_CORES):
        Y += results[c]["yp"].astype(np.float32)
    h = np.maximum(Y + np.asarray(b1, np.float32), np.float32(0.0))
    out = h @ np.asarray(W2, np.float32) + np.asarray(b2, np.float32)
    return out.astype(np.float32)


def kernel(image_embeds, text_embeds, W1, b1, W2, b2):
    in_maps = make_in_maps(image_embeds, text_embeds, W1)
    res = run_device(in_maps, trace=False)
    return finish_host(res.results, b1, W2, b2)


# revision 19
# speedup vs baseline: 1.0169x; 1.0169x over previous
"""Trainium2 Bass kernel for nn_CLIPCrossProductClassifier.

Math:  y[b,h] = sum_{i,j} img_n[b,i] * txt_n[b,j] * W1r[i,j,h]
       logits = relu(y + b1) @ W2 + b2
with img_n/txt_n the L2-normalized embeddings and W1r = W1.reshape(D,D,H).

Sharding: contraction-parallel over i (rows of the bilinear form). Each of
the 8 cores owns 64 values of i (a [64*D, H] row-slice of W1, 32 MB in f16)
and computes a partial y_c[b,h] = sum_{i in I_c, j} ... . The partials are
summed on the host (8 x 1 MB), followed by the tiny bias/ReLU/[512x1]
projection (0.5 MFLOP of the 137 GFLOP total).

Per-core device schedule (b on PSUM partitions so the img scale is a
per-partition scalar):
  for i in 64:                       # i local to the core
    for b_blk in 4:                  # batch in blocks of 128
      psum[b,h] = sum_{j_chunk in 4} txtT[j_chunk, b_blk].T @ W1[i, j_chunk, :, :]
      acc[b_blk] = psum * img[b_blk, i] + acc[b_blk]   # one fused DVE op
W1 streams through as the moving operand in f16 (1 cycle/row at N=512 =
full PE rate, half the HBM bytes of f32); txtT is the stationary operand
(f16 -> fast weight load). W1 is pre-scaled by 512 on the host (sigma ~1,
no f16 subnormals); the 1/512 is folded into the per-partition img scale.

DMA layout: every SBUF row is >= 2 KB so the DMA moves full-size packets
(the f32 baseline was packet-rate-bound at ~1 KB/packet):
 - W1 per i as one [128, 2048] f16 tile (row r = W1[i, {r,128+r,256+r,384+r}, :])
 - txt as one [128, 2048] f16 tile (row r = txtT[{r,128+r,256+r,384+r}, :])
 - img as one [128, 256] f32 tile (row r = img[{r,128+r,256+r,384+r}, :]/512)

Epilogue is a single fused InstTensorScalarPtr per (i, b_blk) on the Vector
engine: acc = (psum * img_scalar) + acc, reading PSUM directly.

Startup: dummy matmuls on a zeroed tile keep the PE HAM un-throttled
(2.4 GHz) while the first input DMAs land; txt is first in the DMA queues.
"""

import numpy as np

import concourse.bass as bass
import concourse.tile as tile
from concourse import bacc, mybir
from concourse.bass_utils import run_bass_kernel_spmd

B, D, H = 512, 512, 512
N_CORES = 8
I_PER_CORE = D // N_CORES          # 64
N_BBLK = B // 128                  # 4
N_JCHUNK = D // 128                # 4
EPS = 1e-12
W1_SCALE = 512.0                   # power of two: exact in fp, folded into img

F32 = mybir.dt.float32
F16 = mybir.dt.float16

N_WARM = 12                        # dummy matmuls to keep HAM warm at start

_CACHE = {}


def _l2norm(x: np.ndarray) -> np.ndarray:
    n = np.sqrt(np.sum(x * x, axis=1, keepdims=True, dtype=np.float32))
    return (x / np.maximum(n, np.float32(EPS))).astype(np.float32)


def build_nc():
    """Build the per-core Bass program (SPMD: same program, per-core data)."""
    nc = bacc.Bacc(
        "TRN2",
        target_bir_lowering=False,
        debug=False,
        num_devices=N_CORES,
    )

    txt_p = nc.dram_tensor("txt_p", [128, N_JCHUNK * B], F16, kind="ExternalInput").ap()
    img_p = nc.dram_tensor(
        "img_p", [128, N_BBLK * I_PER_CORE], F32, kind="ExternalInput"
    ).ap()
    w1_s = nc.dram_tensor(
        "w1_s", [I_PER_CORE, 128, N_JCHUNK * H], F16, kind="ExternalInput"
    ).ap()
    yp = nc.dram_tensor("yp", [B, H], F16, kind="ExternalOutput").ap()

    with tile.TileContext(nc) as tc:
        with (
            tc.tile_pool(name="warm", bufs=1) as warmp,
            tc.tile_pool(name="warmps", bufs=2, space=bass.MemorySpace.PSUM) as warmpsp,
            tc.tile_pool(name="const", bufs=1) as constp,
            tc.tile_pool(name="w1", bufs=8) as w1p,
            tc.tile_pool(name="accs", bufs=1) as accp,
            tc.tile_pool(name="ps", bufs=6, space=bass.MemorySpace.PSUM) as psump,
        ):
            # txt stationary first in the DMA queues: the first matmuls need
            # it before any W1. Halves (2 KB rows) so the first matmul group
            # only waits for chunks c0/c1 + the first W1 half.
            txt_sb = constp.tile([128, N_JCHUNK * B], F16, tag="txt", name="txt_sb")
            w1t0 = w1p.tile([128, N_JCHUNK * H], F16, tag="w1", name="w1p0")
            nc.sync.dma_start(txt_sb[:, : 2 * B], txt_p[:, : 2 * B])
            nc.sync.dma_start(w1t0[:, : 2 * H], w1_s[0, :, : 2 * H])
            nc.sync.dma_start(txt_sb[:, 2 * B :], txt_p[:, 2 * B :])
            nc.sync.dma_start(w1t0[:, 2 * H :], w1_s[0, :, 2 * H :])

            img_sb = constp.tile(
                [128, N_BBLK * I_PER_CORE], F32, tag="img", name="img_sb"
            )
            nc.sync.dma_start(img_sb[:], img_p[:, :])

            acc = [
                accp.tile([128, H], F32, tag=f"acc{bb}", name=f"acc{bb}")
                for bb in range(N_BBLK)
            ]
            # f16 staging for the final partial: halves the output DMA and
            # keeps the f32 accumulator out of the tail critical path.
            yp_sb = [
                accp.tile([128, H], F16, tag=f"yp{bb}", name=f"yp{bb}")
                for bb in range(N_BBLK)
            ]

            # Warm-up: dummy matmuls on a zeroed tile keep the PE busy (and
            # the HAM un-throttled) while the real input DMAs land. They sit
            # ahead of the real matmuls on the tensor queue and have no data
            # dependencies beyond the one memset.
            wz = warmp.tile([128, 384], F16, tag="wz", name="warm_zero")
            nc.vector.memset(wz[:], 0.0)
            for k in range(N_WARM):
                wps = warmpsp.tile([128, 256], F32, tag="wps")
                nc.tensor.matmul(
                    wps[:], wz[:, :128], wz[:, 128:384], start=True, stop=True
                )

            for i in range(I_PER_CORE):
                if i == 0:
                    w1t = w1t0
                else:
                    w1t = w1p.tile([128, N_JCHUNK * H], F16, tag="w1", name="w1")
                    nc.sync.dma_start(w1t[:], w1_s[i])
                for bb in range(N_BBLK):
                    ps = psump.tile([128, H], F32, tag="ps")
                    for c in range(N_JCHUNK):
                        nc.tensor.matmul(
                            ps[:],
                            txt_sb[:, c * B + bb * 128 : c * B + (bb + 1) * 128],
                            w1t[:, c * H : (c + 1) * H],
                            start=(c == 0),
                            stop=(c == N_JCHUNK - 1),
                        )
                    sc = img_sb[:, bb * I_PER_CORE + i : bb * I_PER_CORE + i + 1]
                    if i == 0:
                        # acc = psum * img  (fused, Vector engine)
                        nc.vector.tensor_scalar_mul(acc[bb][:], ps[:], sc)
                    elif i == I_PER_CORE - 1:
                        # Final partial goes straight to the f16 staging tile.
                        nc.vector.scalar_tensor_tensor(
                            yp_sb[bb][:], ps[:], sc, acc[bb][:],
                            mybir.AluOpType.mult, mybir.AluOpType.add,
                        )
                        nc.sync.dma_start(
                            yp[bb * 128 : (bb + 1) * 128, :], yp_sb[bb][:]
                        )
                    else:
                        # acc = (psum * img) + acc  (one fused DVE op)
                        nc.vector.scalar_tensor_tensor(
                            acc[bb][:], ps[:], sc, acc[bb][:],
                            mybir.AluOpType.mult, mybir.AluOpType.add,
                        )

    nc.compile()
    return nc


def make_in_maps(image_embeds, text_embeds, W1):
    imgn = _l2norm(np.asarray(image_embeds, np.float32)) * np.float32(1.0 / W1_SCALE)
    txtn = _l2norm(np.asarray(text_embeds, np.float32))
    # txt packed: row r = [txtT[r], txtT[128+r], txtT[256+r], txtT[384+r]]
    txt_t = np.ascontiguousarray(txtn.T).astype(np.float16)          # [D, B]
    txt_p = np.ascontiguousarray(
        txt_t.reshape(N_JCHUNK, 128, B).transpose(1, 0, 2).reshape(128, N_JCHUNK * B)
    )
    W1r = (np.asarray(W1, np.float32).reshape(D, D, H) * np.float32(W1_SCALE)).astype(
        np.float16
    )
    in_maps = []
    for c in range(N_CORES):
        # img packed per core: row r = [img[r, Ic], img[128+r, Ic], ...]
        ic = np.ascontiguousarray(imgn[:, c * I_PER_CORE : (c + 1) * I_PER_CORE])
        img_pk = np.ascontiguousarray(
            ic.reshape(N_BBLK, 128, I_PER_CORE)
            .transpose(1, 0, 2)
            .reshape(128, N_BBLK * I_PER_CORE)
        )
        # W1 per i: row r = [W1[i, r, :], W1[i, 128+r, :], W1[i, 256+r, :], W1[i, 384+r, :]]
        w1c = W1r[c * I_PER_CORE : (c + 1) * I_PER_CORE]             # [64, D, H]
        w1pk = np.ascontiguousarray(
            w1c.reshape(I_PER_CORE, N_JCHUNK, 128, H)
            .transpose(0, 2, 1, 3)
            .reshape(I_PER_CORE, 128, N_JCHUNK * H)
        )
        in_maps.append({"txt_p": txt_p, "img_p": img_pk, "w1_s": w1pk})
    return in_maps


def run_device(in_maps, trace=False, **kw):
    if "nc" not in _CACHE:
        _CACHE["nc"] = build_nc()
    return run_bass_kernel_spmd(
        _CACHE["nc"], in_maps, list(range(N_CORES)), trace=trace, **kw
    )


def finish_host(results, b1, W2, b2):
    Y = np.zeros((B, H), np.float32)
    for c in range(N_CORES):
        Y += results[c]["yp"].astype(np.float32)
    h = np.maximum(Y + np.asarray(b1, np.float32), np.float32(0.0))
    out = h @ np.asarray(W2, np.float32) + np.asarray(b2, np.float32)
    return out.astype(np.float32)


def kernel(image_embeds, text_embeds, W1, b1, W2, b2):
    in_maps = make_in_maps(image_embeds, text_embeds, W1)
    try:
        res = run_device(in_maps, trace=False)
    except Exception:
        # The axon/PJRT transport occasionally hiccups; one retry recovers.
        res = run_device(in_maps, trace=False)
    return finish_host(res.results, b1, W2, b2)
